# revision 75
# baseline (speedup 1.0000x reference)
"""Trainium2 Bass kernel for nn_LossModule_58213986730076 (loss_fn).

Loss = Ju (contrastive vs N negatives) + Jt (focal triplet over top-8
smallest g) + 1e-3 * ||F F^T - I||_F^2.

Strategy (8 NeuronCores, data-parallel over B):
  - B=8192 rows sharded 1024/core; negatives [N,D] and F [K,D] replicated.
  - All big matmuls in fp8(e4m3) DoubleRow perf mode (0.5 cyc/col, 256-deep
    contraction per instruction); per-column constants (1-nn[n], -fn[k])
    enter via a 1-row bf16 matmul accumulated into the same PSUM group;
    the per-row term pb2[b] = td - ||vhat||^2 rides as the per-partition
    bias/scalar of the relu consumers.
  - Ju relu+rowsum over S''[128,2048] split Act(1536)/DVE(512) with fused
    bias+relu+accum single ops (GPSIMD cannot read PSUM); the Act chunk
    alternates between two PSUM rings so relu(t) overlaps matmul(t+1).
  - Jt: hardware top-8 via DVE max8 on host-negated g (bf16); per-row
    scalars (s, 1/s, threshold) as tiny per-btile ops; m_t via one Act
    Square with per-partition scale; the {0,1} non-selected kill mask
    (Pool is_lt) and m_t are accumulated into the Z psum by identity
    matmuls (I and -BIG*I) on the otherwise idle PE, so the whole Jt
    finalize is one fused DVE relu+rowsum pass (the add+max must be a
    scalar_tensor_tensor against a zeros tensor -- the two-scalar-op
    tensor_scalar form miscompiles in this program).
  - ortho: gram rows sharded 64/core, bf16 matmul emitted between
    b-tiles 6 and 7 into the idle even chA psum ring so its Act Square
    stays off the tail.
  - PE p-state warm-up: dummy matmuls during the DMA fill so real
    matmuls start at 2.4GHz instead of ramping from 0.65GHz.
  - DMAs packed aggressively (HWDGE descriptor generation is ~625ns per
    DMA op, serial across queues): fp8 constants in 3 blocks, one bf16
    gram+identity+mask block, one row block, pair-interleaved b-tile
    input blocks, one output block.
Host does layout only (transposes, dtype casts, replicated norm rows);
all per-sample arithmetic runs on device.
"""

import numpy as np
import ml_dtypes

import concourse.bass as bass
import concourse.bacc as bacc
import concourse.tile as tile
from concourse import mybir
from concourse.bass_utils import run_bass_kernel_spmd

F32 = mybir.dt.float32
BF16 = mybir.dt.bfloat16
FP8 = mybir.dt.float8e4
AluOp = mybir.AluOpType
ActFn = mybir.ActivationFunctionType
PM = mybir.MatmulPerfMode

B, D, N, K, T = 8192, 256, 2048, 512, 8
NCORES = 8
BL = B // NCORES            # 1024 rows per core
P = 128
NBT = BL // P               # 8 b-tiles per core
KSL = K // NCORES           # 64 gram rows per core
LAMBDA_ORTHO = 1e-3

OUT_COLS = 2 * NBT + NBT + 1   # 16 ju cols + 8 jt cols + 1 ortho col

# relu split of the 2048 S'' columns: [Act | DVE]  (GPSIMD can't read PSUM)
CA, CB = 1536, 512

# packed fp8 DoubleRow block: [p, j, 0:1024]=vhatT, [1024:3072]=2*negT,
# [3072:3584]=2*FT   (contract row = p + 128j)
C8W = BL + N + K
# packed bf16 gram block: [p, i, 0:512]=F.T chunk i, [512:576]=F.T slice,
# [576:704]= I (j=0) / -BIG*I (j=1), [704:712]= mask[p,t] (j=0)
CBW = K + KSL + P + NBT
BIGK = 262144.0
# packed bf16 row block: [0, 0:2048]=1-nn, [2048:2560]=-fn
CRW = N + K


def _build_program():
    nc = bacc.Bacc(
        "TRN2", target_bir_lowering=False, debug=False, num_devices=NCORES)
    d_c8 = nc.dram_tensor("c8", [P, 2, C8W], FP8, kind="ExternalInput")
    d_cbf = nc.dram_tensor("cbf", [P, 2, CBW], BF16, kind="ExternalInput")
    d_crow = nc.dram_tensor("crow", [1, CRW], BF16, kind="ExternalInput")
    d_bin = nc.dram_tensor("bin", [P, NBT * (K + 2 * D)], BF16,
                           kind="ExternalInput")
    d_out = nc.dram_tensor("out", [P, OUT_COLS], F32, kind="ExternalOutput")

    with tile.TileContext(nc) as tc:
        with (
            tc.tile_pool(name="const", bufs=1) as cpool,
            tc.tile_pool(name="bin", bufs=3) as binpool,
            tc.tile_pool(name="wk", bufs=3) as wpool,
            tc.tile_pool(name="acc", bufs=1) as apool,
            tc.tile_pool(name="psAe", bufs=1, space="PSUM") as psAe,
            tc.tile_pool(name="psAo", bufs=1, space="PSUM") as psAo,
            tc.tile_pool(name="psB", bufs=1, space="PSUM") as psB,
            tc.tile_pool(name="psZ", bufs=1, space="PSUM") as psZ,
        ):
            # ---------------- accumulators ----------------
            juacc = apool.tile([P, 2 * NBT], F32, name="juacc", tag="juacc")
            jtacc = apool.tile([P, NBT], F32, name="jtacc", tag="jtacc")
            mx8all = apool.tile([P, NBT * 8], F32, name="mx8all", tag="mx8all")
            pb2 = apool.tile([P, NBT], F32, name="pb2", tag="pb2")
            ssum = apool.tile([P, NBT], F32, name="ssum", tag="ssum")
            rn = apool.tile([P, NBT], F32, name="rn", tag="rn")
            maskc = apool.tile([P, NBT], F32, name="maskc", tag="maskc")
            obuf = apool.tile([P, OUT_COLS], F32, name="obuf", tag="obuf")

            # ------- startup DMAs spread across the four DGE queues -------
            # (each engine's hardware descriptor generator is ~625ns per DMA
            # op, serial per queue; the critical path is the first S'' group:
            # vhatT + negT first half + srow)
            c8 = cpool.tile([P, 2, C8W], FP8, name="c8", tag="c8")
            crow = cpool.tile([1, CRW], BF16, name="crow", tag="crow")
            cbf = cpool.tile([P, 2, CBW], BF16, name="cbf", tag="cbf")
            nc.scalar.dma_start(c8[:, :, 0:BL], d_c8[:, :, 0:BL])
            nc.gpsimd.dma_start(c8[:, :, BL:BL + CA],
                                d_c8[:, :, BL:BL + CA])
            nc.gpsimd.dma_start(c8[:, :, BL + CA:],
                                d_c8[:, :, BL + CA:])
            ones = cpool.tile([1, P], BF16, name="ones", tag="ones")
            nc.vector.memset(ones[:], 1.0)
            zerot = cpool.tile([P, K], BF16, name="zerot", tag="zerot")
            nc.vector.memset(zerot[:], 0.0)
            # PE p-state warm-up: ~4us of dummy matmuls while DMAs stream in
            # (the cost model runs the PE at 0.65/1.2GHz until it has been
            # continuously busy for 3us; these make the first real matmuls
            # run at the full 2.4GHz)
            warm = psB.tile([P, CB], F32, name="warm", tag="chB")
            for _ in range(8):
                nc.tensor.matmul(warm[:], zerot[:, 0:P], zerot[:],
                                 start=True, stop=True)

            # SP queue: b-tile input pairs first, the gram block (needed
            # late) at the back.  HWDGE descriptor generation is serial
            # across ALL queues, so fewer DMA ops = faster pipeline fill.
            BW = K + 2 * D
            def bin_pair(i):
                bp = binpool.tile([P, 2, BW], BF16, name="bp", tag="bp")
                nc.sync.dma_start(bp[:], d_bin[:, 2 * BW * i:2 * BW * (i + 1)])
                return bp
            pairs = [bin_pair(0)]
            nc.sync.dma_start(crow[:], d_crow[:])
            pairs.append(bin_pair(1))
            pairs.append(bin_pair(2))
            nc.sync.dma_start(cbf[:], d_cbf[:])

            def vh_sl(t):
                return c8[:, :, bass.ts(t, P)]


            ident = cbf[:, 0, K + KSL:K + KSL + P]
            identneg = cbf[:, 1, K + KSL:K + KSL + P]
            neg0 = BL
            fil = c8[:, :, neg0 + N:]
            srow = crow[:, 0:N]
            zrow = crow[:, N:]

            # ---------------- per b-tile pipeline ----------------
            # the pb2 prep (Pool w1/w2) runs one b-tile ahead so the Act
            # relu's bias is ready the moment the matmuls drain
            def get_bin(t):
                if t // 2 >= len(pairs):
                    pairs.append(bin_pair(t // 2))
                return pairs[t // 2][:, t % 2, :]

            def w2_prep(bin_t):
                vt = bin_t[:, K:K + D]
                vht = bin_t[:, K + D:]
                w1 = wpool.tile([P, D], BF16, name="w1", tag="w1")
                nc.gpsimd.tensor_scalar(w1[:], vht[:], -2.0, None,
                                        op0=AluOp.mult)
                w2 = wpool.tile([P, D], BF16, name="w2", tag="w2")
                nc.gpsimd.tensor_tensor(w2[:], w1[:], vt[:], op=AluOp.add)
                return w2

            bin_next = get_bin(0)
            w2_next = w2_prep(bin_next)
            for t in range(NBT):
                bin_t = bin_next
                w2 = w2_next
                gneg = bin_t[:, 0:K]
                vt = bin_t[:, K:K + D]

                # pb2 = sum(v*(v - 2*vhat))   (= td - ||vhat||^2)
                q1 = wpool.tile([P, D], BF16, name="q1", tag="q1")
                nc.vector.scalar_tensor_tensor(
                    q1[:], vt[:], 1.0, w2[:], op0=AluOp.mult,
                    op1=AluOp.mult, accum_out=pb2[:, t:t + 1])
                pcol = pb2[:, t:t + 1]

                # top-8 smallest g = top-8 largest gneg
                nc.vector.max(out=mx8all[:, 8 * t:8 * t + 8], in_=gneg)
                nc.vector.tensor_reduce(
                    ssum[:, t:t + 1], mx8all[:, 8 * t:8 * t + 8],
                    axis=mybir.AxisListType.X, op=AluOp.add, negate=True)
                nc.vector.reciprocal(rn[:, t:t + 1], ssum[:, t:t + 1])

                # next b-tile's input + pb2 prep (Pool)
                if t + 1 < NBT:
                    bin_next = get_bin(t + 1)
                    w2_next = w2_prep(bin_next)

                # kill mask {0,1} at non-selected (SBUF -> Pool)
                kil = wpool.tile([P, K], BF16, name="kil", tag="kil")
                nc.gpsimd.tensor_scalar(kil[:], gneg,
                                        mx8all[:, 8 * t + 7:8 * t + 8], None,
                                        op0=AluOp.is_lt)
                mtb = wpool.tile([P, K], BF16, name="mtb", tag="mtb")
                nc.scalar.activation(mtb[:], gneg, ActFn.Square,
                                     bias=1.0, scale=rn[:, t:t + 1])

                # matmuls: S'' chunks first (feed the relu consumers), Z last
                vsl = vh_sl(t)
                psA = psAe if t % 2 == 0 else psAo
                chA = psA.tile([P, CA], F32, name="chA", tag="chA")
                for lo in range(0, CA, 512):
                    hi = min(lo + 512, CA)
                    po = chA[:, lo:hi]
                    nc.tensor.matmul(po, vsl, c8[:, :, neg0 + lo:neg0 + hi],
                                     start=True, stop=False,
                                     perf_mode=PM.DoubleRow)
                    nc.tensor.matmul(po, ones[:], srow[:, lo:hi],
                                     start=False, stop=True)
                chB = psB.tile([P, CB], F32, name="chB", tag="chB")
                nc.tensor.matmul(chB[:], vsl,
                                 c8[:, :, neg0 + CA:neg0 + N],
                                 start=True, stop=False,
                                 perf_mode=PM.DoubleRow)
                nc.tensor.matmul(chB[:], ones[:], srow[:, CA:],
                                 start=False, stop=True)
                # Z psum accumulates the whole Jt relu argument (minus the
                # per-row pb2): 2vh.F - fn + m_t + kill, the last two added
                # by identity matmuls on the otherwise idle PE
                zp = psZ.tile([P, K], F32, name="zp", tag="zp")
                nc.tensor.matmul(zp[:], vsl, fil, start=True, stop=False,
                                 perf_mode=PM.DoubleRow)
                nc.tensor.matmul(zp[:], ident, mtb[:], start=False,
                                 stop=False)
                nc.tensor.matmul(zp[:], identneg, kil[:], start=False,
                                 stop=False)
                nc.tensor.matmul(zp[:], ones[:], zrow, start=False,
                                 stop=True)

                # Ju relu(bias + .) + rowsum on Act / DVE (drain psum early)
                nc.scalar.activation(chA[:], chA[:], ActFn.Relu, bias=pcol,
                                     accum_out=juacc[:, t:t + 1])
                nc.vector.tensor_scalar(chB[:], chB[:], pcol, 0.0,
                                        op0=AluOp.add, op1=AluOp.max,
                                        accum_out=juacc[:, NBT + t:NBT + t + 1])

                if t == NBT - 2:
                    # ortho partial: use the even chA psum ring (free after
                    # relu(6)); its Act Square then runs before relu(7)
                    # instead of serializing after the last b-tile
                    gram = psAe.tile([P, CA], F32, name="gram", tag="chA")
                    oacc = obuf[0:KSL, 3 * NBT:3 * NBT + 1]
                    nc.tensor.matmul(gram[0:KSL, 0:K],
                                     cbf[:, 0, K:K + KSL], cbf[:, 0, 0:K],
                                     start=True, stop=False)
                    nc.tensor.matmul(gram[0:KSL, 0:K],
                                     cbf[:, 1, K:K + KSL], cbf[:, 1, 0:K],
                                     start=False, stop=True)
                    gsq = wpool.tile([KSL, K], BF16, name="gsq", tag="gsq")
                    nc.scalar.activation(gsq[:], gram[0:KSL, 0:K],
                                         ActFn.Square, accum_out=oacc)

                # Jt: relu(zp + pb2) + rowsum in one DVE pass
                zro = wpool.tile([P, K], BF16, name="zro", tag="zro")
                nc.vector.scalar_tensor_tensor(
                    zro[:], zp[:], pcol, zerot[:],
                    op0=AluOp.add, op1=AluOp.max,
                    accum_out=jtacc[:, t:t + 1])

            nc.vector.tensor_copy(maskc[:], cbf[:, 0, K + KSL + P:])

            # ---- apply mask into the output block, single DMA out ----
            nc.vector.tensor_tensor(obuf[:, 2 * NBT:3 * NBT], jtacc[:],
                                    maskc[:], op=AluOp.mult)
            for i in range(2):
                nc.vector.tensor_tensor(
                    obuf[:, i * NBT:(i + 1) * NBT],
                    juacc[:, i * NBT:(i + 1) * NBT],
                    maskc[:], op=AluOp.mult)
            nc.sync.dma_start(d_out[:], obuf[:])

    nc.compile()
    return nc


_PROGRAM = None


def _get_program():
    global _PROGRAM
    if _PROGRAM is None:
        _PROGRAM = _build_program()
    return _PROGRAM


def _host_prep(v, vhat, g, F, negatives, mask):
    """Per-core input layout. Only layout transforms + dtype casts +
    replicated constant norm rows happen here; all per-sample math runs
    on device."""
    f64 = np.float64
    bf16 = ml_dtypes.bfloat16
    fp8 = ml_dtypes.float8_e4m3

    def to_fp8(x):
        return np.clip(x, -240.0, 240.0).astype(fp8)

    # fp8 DoubleRow block: [p, j, col] with contract row = p + 128j
    c8 = np.empty((P, 2, C8W), dtype=fp8)
    vhT = vhat.T.reshape(2, P, B).transpose(1, 0, 2)     # [p, j, b]
    c8_neg = to_fp8((2.0 * negatives.T).reshape(2, P, N).transpose(1, 0, 2))
    c8_fil = to_fp8((2.0 * F.T).reshape(2, P, K).transpose(1, 0, 2))
    c8[:, :, BL + 0:BL + N] = c8_neg
    c8[:, :, BL + N:] = c8_fil

    nn = (negatives.astype(f64) ** 2).sum(axis=1)
    fn = (F.astype(f64) ** 2).sum(axis=1)
    crow = np.empty((1, CRW), dtype=bf16)
    crow[0, 0:N] = (1.0 - nn).astype(bf16)
    crow[0, N:] = (-fn).astype(bf16)

    # bf16 gram block: [p, i, 0:512] = F.T chunk i; [512:576] per-core slice
    fT_bf = F.T.astype(bf16)                              # [D, K]
    cbf = np.zeros((P, 2, CBW), dtype=bf16)
    cbf[:, 0, 0:K] = fT_bf[0:P]
    cbf[:, 1, 0:K] = fT_bf[P:]
    cbf[:, 0, K + KSL:K + KSL + P] = np.eye(P)
    cbf[:, 1, K + KSL:K + KSL + P] = -BIGK * np.eye(P)

    # per-row packed input block [gneg | v | vh], pair-of-btile interleaved:
    # bin[p, ((i*2+j)*BW + c)] = block[256*i + 128*j + p, c]
    BW = K + 2 * D
    binb = np.empty((B, BW), dtype=bf16)
    binb[:, 0:K] = -g
    binb[:, K:K + D] = v
    binb[:, K + D:] = vhat

    # [p, t] = mask[bs + t*128 + p]
    maskf = mask.astype(np.float32).reshape(NCORES, NBT, P).transpose(0, 2, 1)

    vhT8 = to_fp8(vhT)
    in_maps = []
    for c in range(NCORES):
        bs = slice(c * BL, (c + 1) * BL)
        c8c = c8.copy()
        c8c[:, :, 0:BL] = vhT8[:, :, bs]
        cbfc = cbf.copy()
        cbfc[:, 0, K:K + KSL] = fT_bf[0:P, c * KSL:(c + 1) * KSL]
        cbfc[:, 1, K:K + KSL] = fT_bf[P:, c * KSL:(c + 1) * KSL]
        cbfc[:, 0, K + KSL + P:] = maskf[c]
        in_maps.append({
            "c8": c8c,
            "cbf": cbfc,
            "crow": crow,
            "bin": np.ascontiguousarray(
                binb[bs].reshape(NBT // 2, 2, P, BW)
                .transpose(2, 0, 1, 3).reshape(P, NBT * BW)),
        })
    return in_maps, fn


def _host_combine(results, fn, mask):
    jusum = 0.0
    jtsum = 0.0
    osum = 0.0
    for r in results:
        out = np.asarray(r["out"], dtype=np.float64)
        jusum += out[:, 0:2 * NBT].sum()
        jtsum += out[:, 2 * NBT:3 * NBT].sum()
        osum += out[0:KSL, 3 * NBT].sum()

    msum = float(mask.astype(np.float64).sum())
    if msum == 0.0:
        Ju = 0.0
        Jt = 0.0
    else:
        Ju = jusum / (N * msum)
        Jt = jtsum / msum
    ortho_sq = osum - 2.0 * float(fn.sum()) + float(K)
    Jz = Ju + Jt + LAMBDA_ORTHO * ortho_sq
    return np.float32(Jz)


def kernel(v, vhat, g, F, negatives, mask, **run_kwargs):
    nc = _get_program()
    in_maps, fn = _host_prep(
        np.asarray(v, dtype=np.float32), np.asarray(vhat, dtype=np.float32),
        np.asarray(g, dtype=np.float32), np.asarray(F, dtype=np.float32),
        np.asarray(negatives, dtype=np.float32), np.asarray(mask))
    res = run_bass_kernel_spmd(nc, in_maps, core_ids=list(range(NCORES)),
                               **run_kwargs)
    out = _host_combine(res.results, fn, np.asarray(mask))
    if run_kwargs:
        return out, res
    return out


# revision 76
# speedup vs baseline: 1.0226x; 1.0226x over previous
"""Trainium2 Bass kernel for nn_LossModule_58213986730076 (loss_fn).

Loss = Ju (contrastive vs N negatives) + Jt (focal triplet over top-8
smallest g) + 1e-3 * ||F F^T - I||_F^2.

Strategy (8 NeuronCores, data-parallel over B):
  - B=8192 rows sharded 1024/core; negatives [N,D] and F [K,D] replicated.
  - All big matmuls in fp8(e4m3) DoubleRow perf mode (0.5 cyc/col, 256-deep
    contraction per instruction); per-column constants (1-nn[n], -fn[k])
    enter via a 1-row bf16 matmul accumulated into the same PSUM group;
    the per-row term pb2[b] = td - ||vhat||^2 rides as the per-partition
    bias/scalar of the relu consumers.
  - Ju relu+rowsum over S''[128,2048] split Act(1536)/DVE(512) with fused
    bias+relu+accum single ops (GPSIMD cannot read PSUM); the Act chunk
    alternates between two PSUM rings so relu(t) overlaps matmul(t+1).
  - Jt: hardware top-8 via DVE max8 on host-negated g (bf16); per-row
    scalars (s, 1/s, threshold) as tiny per-btile ops; m_t via one Act
    Square with per-partition scale; the {0,1} non-selected kill mask
    (Pool is_lt) and m_t are accumulated into the Z psum by identity
    matmuls (I and -BIG*I) on the otherwise idle PE, so the whole Jt
    finalize is one fused DVE relu+rowsum pass (the add+max must be a
    scalar_tensor_tensor against a zeros tensor -- the two-scalar-op
    tensor_scalar form miscompiles in this program).
  - ortho: gram rows sharded 64/core, bf16 matmul emitted between
    b-tiles 6 and 7 into the idle even chA psum ring so its Act Square
    stays off the tail.
  - PE p-state warm-up: dummy matmuls during the DMA fill so real
    matmuls start at 2.4GHz instead of ramping from 0.65GHz.
  - DMAs packed aggressively (HWDGE descriptor generation is ~625ns per
    DMA op, serial across queues): fp8 constants in 3 blocks, one bf16
    gram+identity+mask block, one row block, pair-interleaved b-tile
    input blocks, one output block.
Host does layout only (transposes, dtype casts, replicated norm rows);
all per-sample arithmetic runs on device.
"""

import numpy as np
import ml_dtypes

import concourse.bass as bass
import concourse.bacc as bacc
import concourse.tile as tile
from concourse import mybir
from concourse.bass_utils import run_bass_kernel_spmd

F32 = mybir.dt.float32
BF16 = mybir.dt.bfloat16
FP8 = mybir.dt.float8e4
AluOp = mybir.AluOpType
ActFn = mybir.ActivationFunctionType
PM = mybir.MatmulPerfMode

B, D, N, K, T = 8192, 256, 2048, 512, 8
NCORES = 8
BL = B // NCORES            # 1024 rows per core
P = 128
NBT = BL // P               # 8 b-tiles per core
KSL = K // NCORES           # 64 gram rows per core
LAMBDA_ORTHO = 1e-3

OUT_COLS = 2 * NBT + NBT + 1   # 16 ju cols + 8 jt cols + 1 ortho col

# relu split of the 2048 S'' columns: [Act | DVE]  (GPSIMD can't read PSUM)
CA, CB = 1536, 512

# packed fp8 DoubleRow block: [p, j, 0:1024]=vhatT, [1024:3072]=2*negT,
# [3072:3584]=2*FT   (contract row = p + 128j)
C8W = BL + N + K
# packed bf16 gram block: [p, i, 0:512]=F.T chunk i, [512:576]=F.T slice,
# [576:704]= I (j=0) / -BIG*I (j=1), [704:712]= mask[p,t] (j=0)
CBW = K + KSL + P + NBT
BIGK = 262144.0
# packed bf16 row block: [0, 0:2048]=1-nn, [2048:2560]=-fn
CRW = N + K


def _build_program():
    nc = bacc.Bacc(
        "TRN2", target_bir_lowering=False, debug=False, num_devices=NCORES)
    d_c8 = nc.dram_tensor("c8", [P, 2, C8W], FP8, kind="ExternalInput")
    d_cbf = nc.dram_tensor("cbf", [P, 2, CBW], BF16, kind="ExternalInput")
    d_crow = nc.dram_tensor("crow", [1, CRW], BF16, kind="ExternalInput")
    d_bin = nc.dram_tensor("bin", [P, NBT * (K + 2 * D)], BF16,
                           kind="ExternalInput")
    d_out = nc.dram_tensor("out", [P, OUT_COLS], F32, kind="ExternalOutput")

    with tile.TileContext(nc) as tc:
        with (
            tc.tile_pool(name="const", bufs=1) as cpool,
            tc.tile_pool(name="bin", bufs=3) as binpool,
            tc.tile_pool(name="wk", bufs=3) as wpool,
            tc.tile_pool(name="acc", bufs=1) as apool,
            tc.tile_pool(name="psAe", bufs=1, space="PSUM") as psAe,
            tc.tile_pool(name="psAo", bufs=1, space="PSUM") as psAo,
            tc.tile_pool(name="psB", bufs=1, space="PSUM") as psB,
            tc.tile_pool(name="psZ", bufs=1, space="PSUM") as psZ,
        ):
            # ---------------- accumulators ----------------
            juacc = apool.tile([P, 2 * NBT], F32, name="juacc", tag="juacc")
            jtacc = apool.tile([P, NBT], F32, name="jtacc", tag="jtacc")
            mx8all = apool.tile([P, NBT * 8], F32, name="mx8all", tag="mx8all")
            pb2 = apool.tile([P, NBT], F32, name="pb2", tag="pb2")
            ssum = apool.tile([P, NBT], F32, name="ssum", tag="ssum")
            rn = apool.tile([P, NBT], F32, name="rn", tag="rn")
            maskc = apool.tile([P, NBT], F32, name="maskc", tag="maskc")
            obuf = apool.tile([P, OUT_COLS], F32, name="obuf", tag="obuf")

            # ------- startup DMAs spread across the four DGE queues -------
            # (each engine's hardware descriptor generator is ~625ns per DMA
            # op, serial per queue; the critical path is the first S'' group:
            # vhatT + negT first half + srow)
            c8 = cpool.tile([P, 2, C8W], FP8, name="c8", tag="c8")
            crow = cpool.tile([1, CRW], BF16, name="crow", tag="crow")
            cbf = cpool.tile([P, 2, CBW], BF16, name="cbf", tag="cbf")
            nc.scalar.dma_start(c8[:, :, 0:BL], d_c8[:, :, 0:BL])
            nc.gpsimd.dma_start(c8[:, :, BL:BL + CA],
                                d_c8[:, :, BL:BL + CA])
            nc.gpsimd.dma_start(c8[:, :, BL + CA:],
                                d_c8[:, :, BL + CA:])
            ones = cpool.tile([1, P], BF16, name="ones", tag="ones")
            nc.vector.memset(ones[:], 1.0)
            zerot = cpool.tile([P, K], BF16, name="zerot", tag="zerot")
            nc.vector.memset(zerot[:], 0.0)
            # PE p-state warm-up: ~4us of dummy matmuls while DMAs stream in
            # (the cost model runs the PE at 0.65/1.2GHz until it has been
            # continuously busy for 3us; these make the first real matmuls
            # run at the full 2.4GHz)
            warm = psB.tile([P, CB], F32, name="warm", tag="chB")
            for _ in range(8):
                nc.tensor.matmul(warm[:], zerot[:, 0:P], zerot[:],
                                 start=True, stop=True)

            # SP queue: b-tile input pairs first, the gram block (needed
            # late) at the back.  HWDGE descriptor generation is serial
            # across ALL queues, so fewer DMA ops = faster pipeline fill.
            BW = K + 2 * D
            def bin_pair(i):
                bp = binpool.tile([P, 2, BW], BF16, name="bp", tag="bp")
                nc.sync.dma_start(bp[:], d_bin[:, 2 * BW * i:2 * BW * (i + 1)])
                return bp
            pairs = [bin_pair(0)]
            nc.sync.dma_start(crow[:], d_crow[:])
            pairs.append(bin_pair(1))
            pairs.append(bin_pair(2))
            nc.sync.dma_start(cbf[:], d_cbf[:])

            def vh_sl(t):
                return c8[:, :, bass.ts(t, P)]


            ident = cbf[:, 0, K + KSL:K + KSL + P]
            identneg = cbf[:, 1, K + KSL:K + KSL + P]
            neg0 = BL
            fil = c8[:, :, neg0 + N:]
            srow = crow[:, 0:N]
            zrow = crow[:, N:]

            # ---------------- per b-tile pipeline ----------------
            # the pb2 prep (Pool w1/w2) runs one b-tile ahead so the Act
            # relu's bias is ready the moment the matmuls drain
            def get_bin(t):
                if t // 2 >= len(pairs):
                    pairs.append(bin_pair(t // 2))
                return pairs[t // 2][:, t % 2, :]

            def w2_prep(bin_t):
                vt = bin_t[:, K:K + D]
                vht = bin_t[:, K + D:]
                w1 = wpool.tile([P, D], BF16, name="w1", tag="w1")
                nc.gpsimd.tensor_scalar(w1[:], vht[:], -2.0, None,
                                        op0=AluOp.mult)
                w2 = wpool.tile([P, D], BF16, name="w2", tag="w2")
                nc.gpsimd.tensor_tensor(w2[:], w1[:], vt[:], op=AluOp.add)
                return w2

            def top8_prep(t, bin_t):
                gneg = bin_t[:, 0:K]
                nc.vector.max(out=mx8all[:, 8 * t:8 * t + 8], in_=gneg)
                nc.vector.tensor_reduce(
                    ssum[:, t:t + 1], mx8all[:, 8 * t:8 * t + 8],
                    axis=mybir.AxisListType.X, op=AluOp.add, negate=True)
                nc.vector.reciprocal(rn[:, t:t + 1], ssum[:, t:t + 1])
                mtb = wpool.tile([P, K], BF16, name="mtb", tag="mtb")
                nc.scalar.activation(mtb[:], gneg, ActFn.Square,
                                     bias=1.0, scale=rn[:, t:t + 1])
                return mtb

            bin_next = get_bin(0)
            w2_next = w2_prep(bin_next)
            mtb_next = top8_prep(0, bin_next)
            for t in range(NBT):
                bin_t = bin_next
                w2 = w2_next
                mtb = mtb_next
                gneg = bin_t[:, 0:K]
                vt = bin_t[:, K:K + D]

                # pb2 = sum(v*(v - 2*vhat))   (= td - ||vhat||^2)
                q1 = wpool.tile([P, D], BF16, name="q1", tag="q1")
                nc.vector.scalar_tensor_tensor(
                    q1[:], vt[:], 1.0, w2[:], op0=AluOp.mult,
                    op1=AluOp.mult, accum_out=pb2[:, t:t + 1])
                pcol = pb2[:, t:t + 1]

                # next b-tile's input + prep, one b-tile ahead so Act's
                # m_t Square is queued ready between relus
                if t + 1 < NBT:
                    bin_next = get_bin(t + 1)
                    w2_next = w2_prep(bin_next)
                    mtb_next = top8_prep(t + 1, bin_next)

                # kill mask {0,1} at non-selected (SBUF -> Pool)
                kil = wpool.tile([P, K], BF16, name="kil", tag="kil")
                nc.gpsimd.tensor_scalar(kil[:], gneg,
                                        mx8all[:, 8 * t + 7:8 * t + 8], None,
                                        op0=AluOp.is_lt)

                # matmuls: S'' chunks first (feed the relu consumers), Z last
                vsl = vh_sl(t)
                psA = psAe if t % 2 == 0 else psAo
                chA = psA.tile([P, CA], F32, name="chA", tag="chA")
                for lo in range(0, CA, 512):
                    hi = min(lo + 512, CA)
                    po = chA[:, lo:hi]
                    nc.tensor.matmul(po, vsl, c8[:, :, neg0 + lo:neg0 + hi],
                                     start=True, stop=False,
                                     perf_mode=PM.DoubleRow)
                    nc.tensor.matmul(po, ones[:], srow[:, lo:hi],
                                     start=False, stop=True)
                chB = psB.tile([P, CB], F32, name="chB", tag="chB")
                nc.tensor.matmul(chB[:], vsl,
                                 c8[:, :, neg0 + CA:neg0 + N],
                                 start=True, stop=False,
                                 perf_mode=PM.DoubleRow)
                nc.tensor.matmul(chB[:], ones[:], srow[:, CA:],
                                 start=False, stop=True)
                # Z psum accumulates the whole Jt relu argument (minus the
                # per-row pb2): 2vh.F - fn + m_t + kill, the last two added
                # by identity matmuls on the otherwise idle PE
                zp = psZ.tile([P, K], F32, name="zp", tag="zp")
                nc.tensor.matmul(zp[:], vsl, fil, start=True, stop=False,
                                 perf_mode=PM.DoubleRow)
                nc.tensor.matmul(zp[:], ident, mtb[:], start=False,
                                 stop=False)
                nc.tensor.matmul(zp[:], identneg, kil[:], start=False,
                                 stop=False)
                nc.tensor.matmul(zp[:], ones[:], zrow, start=False,
                                 stop=True)

                # Ju relu(bias + .) + rowsum on Act / DVE (drain psum early)
                nc.scalar.activation(chA[:], chA[:], ActFn.Relu, bias=pcol,
                                     accum_out=juacc[:, t:t + 1])
                nc.vector.tensor_scalar(chB[:], chB[:], pcol, 0.0,
                                        op0=AluOp.add, op1=AluOp.max,
                                        accum_out=juacc[:, NBT + t:NBT + t + 1])

                if t == NBT - 2:
                    # ortho partial: use the even chA psum ring (free after
                    # relu(6)); its Act Square then runs before relu(7)
                    # instead of serializing after the last b-tile
                    gram = psAe.tile([P, CA], F32, name="gram", tag="chA")
                    oacc = obuf[0:KSL, 3 * NBT:3 * NBT + 1]
                    nc.tensor.matmul(gram[0:KSL, 0:K],
                                     cbf[:, 0, K:K + KSL], cbf[:, 0, 0:K],
                                     start=True, stop=False)
                    nc.tensor.matmul(gram[0:KSL, 0:K],
                                     cbf[:, 1, K:K + KSL], cbf[:, 1, 0:K],
                                     start=False, stop=True)
                    gsq = wpool.tile([KSL, K], BF16, name="gsq", tag="gsq")
                    nc.scalar.activation(gsq[:], gram[0:KSL, 0:K],
                                         ActFn.Square, accum_out=oacc)

                # Jt: relu(zp + pb2) + rowsum in one DVE pass
                zro = wpool.tile([P, K], BF16, name="zro", tag="zro")
                nc.vector.scalar_tensor_tensor(
                    zro[:], zp[:], pcol, zerot[:],
                    op0=AluOp.add, op1=AluOp.max,
                    accum_out=jtacc[:, t:t + 1])

            nc.vector.tensor_copy(maskc[:], cbf[:, 0, K + KSL + P:])

            # ---- apply mask into the output block, single DMA out ----
            nc.vector.tensor_tensor(obuf[:, 2 * NBT:3 * NBT], jtacc[:],
                                    maskc[:], op=AluOp.mult)
            for i in range(2):
                nc.vector.tensor_tensor(
                    obuf[:, i * NBT:(i + 1) * NBT],
                    juacc[:, i * NBT:(i + 1) * NBT],
                    maskc[:], op=AluOp.mult)
            nc.sync.dma_start(d_out[:], obuf[:])

    nc.compile()
    return nc


_PROGRAM = None


def _get_program():
    global _PROGRAM
    if _PROGRAM is None:
        _PROGRAM = _build_program()
    return _PROGRAM


def _host_prep(v, vhat, g, F, negatives, mask):
    """Per-core input layout. Only layout transforms + dtype casts +
    replicated constant norm rows happen here; all per-sample math runs
    on device."""
    f64 = np.float64
    bf16 = ml_dtypes.bfloat16
    fp8 = ml_dtypes.float8_e4m3

    def to_fp8(x):
        return np.clip(x, -240.0, 240.0).astype(fp8)

    # fp8 DoubleRow block: [p, j, col] with contract row = p + 128j
    c8 = np.empty((P, 2, C8W), dtype=fp8)
    vhT = vhat.T.reshape(2, P, B).transpose(1, 0, 2)     # [p, j, b]
    c8_neg = to_fp8((2.0 * negatives.T).reshape(2, P, N).transpose(1, 0, 2))
    c8_fil = to_fp8((2.0 * F.T).reshape(2, P, K).transpose(1, 0, 2))
    c8[:, :, BL + 0:BL + N] = c8_neg
    c8[:, :, BL + N:] = c8_fil

    nn = (negatives.astype(f64) ** 2).sum(axis=1)
    fn = (F.astype(f64) ** 2).sum(axis=1)
    crow = np.empty((1, CRW), dtype=bf16)
    crow[0, 0:N] = (1.0 - nn).astype(bf16)
    crow[0, N:] = (-fn).astype(bf16)

    # bf16 gram block: [p, i, 0:512] = F.T chunk i; [512:576] per-core slice
    fT_bf = F.T.astype(bf16)                              # [D, K]
    cbf = np.zeros((P, 2, CBW), dtype=bf16)
    cbf[:, 0, 0:K] = fT_bf[0:P]
    cbf[:, 1, 0:K] = fT_bf[P:]
    cbf[:, 0, K + KSL:K + KSL + P] = np.eye(P)
    cbf[:, 1, K + KSL:K + KSL + P] = -BIGK * np.eye(P)

    # per-row packed input block [gneg | v | vh], pair-of-btile interleaved:
    # bin[p, ((i*2+j)*BW + c)] = block[256*i + 128*j + p, c]
    BW = K + 2 * D
    binb = np.empty((B, BW), dtype=bf16)
    binb[:, 0:K] = -g
    binb[:, K:K + D] = v
    binb[:, K + D:] = vhat

    # [p, t] = mask[bs + t*128 + p]
    maskf = mask.astype(np.float32).reshape(NCORES, NBT, P).transpose(0, 2, 1)

    vhT8 = to_fp8(vhT)
    in_maps = []
    for c in range(NCORES):
        bs = slice(c * BL, (c + 1) * BL)
        c8c = c8.copy()
        c8c[:, :, 0:BL] = vhT8[:, :, bs]
        cbfc = cbf.copy()
        cbfc[:, 0, K:K + KSL] = fT_bf[0:P, c * KSL:(c + 1) * KSL]
        cbfc[:, 1, K:K + KSL] = fT_bf[P:, c * KSL:(c + 1) * KSL]
        cbfc[:, 0, K + KSL + P:] = maskf[c]
        in_maps.append({
            "c8": c8c,
            "cbf": cbfc,
            "crow": crow,
            "bin": np.ascontiguousarray(
                binb[bs].reshape(NBT // 2, 2, P, BW)
                .transpose(2, 0, 1, 3).reshape(P, NBT * BW)),
        })
    return in_maps, fn


def _host_combine(results, fn, mask):
    jusum = 0.0
    jtsum = 0.0
    osum = 0.0
    for r in results:
        out = np.asarray(r["out"], dtype=np.float64)
        jusum += out[:, 0:2 * NBT].sum()
        jtsum += out[:, 2 * NBT:3 * NBT].sum()
        osum += out[0:KSL, 3 * NBT].sum()

    msum = float(mask.astype(np.float64).sum())
    if msum == 0.0:
        Ju = 0.0
        Jt = 0.0
    else:
        Ju = jusum / (N * msum)
        Jt = jtsum / msum
    ortho_sq = osum - 2.0 * float(fn.sum()) + float(K)
    Jz = Ju + Jt + LAMBDA_ORTHO * ortho_sq
    return np.float32(Jz)


def kernel(v, vhat, g, F, negatives, mask, **run_kwargs):
    nc = _get_program()
    in_maps, fn = _host_prep(
        np.asarray(v, dtype=np.float32), np.asarray(vhat, dtype=np.float32),
        np.asarray(g, dtype=np.float32), np.asarray(F, dtype=np.float32),
        np.asarray(negatives, dtype=np.float32), np.asarray(mask))
    res = run_bass_kernel_spmd(nc, in_maps, core_ids=list(range(NCORES)),
                               **run_kwargs)
    out = _host_combine(res.results, fn, np.asarray(mask))
    if run_kwargs:
        return out, res
    return out
